# revision 12
# baseline (speedup 1.0000x reference)
"""CPAttention Trainium2 kernel: 8-way batch-data-parallel over 8 NeuronCores.

v3: single-head attention loop, reduction-free score/Z plumbing.
  - qkproj fp32 (score fidelity), merged 1024-col matmuls
  - dots: fp32 per (head, jt), [128 keys, 1024 queries]
  - score: A_t[jt] += |dots*mask| accumulated over heads on GpSimd
    (fused abs_max/add), reduced by 8 fp32 ones-matmuls at the end
  - Z: rides the AV matmul as a 65th V column (ones); softmax denominator
    extracted from PSUM row 64, broadcast via GpSimd partition_broadcast
    (no PE broadcast, no extra PSUM)
  - AV/projections bf16
Host applies the argsort + 16-step row swap (commutes with w_out).
"""
import numpy as np

import concourse.bacc as bacc
import concourse.tile as tile
from concourse import mybir
from concourse.bass_utils import run_bass_kernel_spmd

F32 = mybir.dt.float32
BF16 = mybir.dt.bfloat16
U32 = mybir.dt.uint32
AOP = mybir.AluOpType
AFT = mybir.ActivationFunctionType

B, N, DIM = 8, 1024, 512
HEADS, DH = 8, 64
INNER = 512
SCALE = DH ** -0.5

_cache = {}


def _build():
    nc = bacc.Bacc()
    xT = nc.declare_dram_parameter("xT", [DIM, N], F32, isOutput=False)
    xTbf = nc.declare_dram_parameter("xTbf", [DIM, N], BF16, isOutput=False)
    maskT = nc.declare_dram_parameter("maskT", [N, N], BF16, isOutput=False)
    wqk = nc.declare_dram_parameter("wqk", [DIM, 2 * INNER], F32, isOutput=False)
    wvbf = nc.declare_dram_parameter("wvbf", [DIM, INNER], BF16, isOutput=False)
    wobf = nc.declare_dram_parameter("wobf", [INNER, DIM], BF16, isOutput=False)
    bout = nc.declare_dram_parameter("bout", [1, DIM], F32, isOutput=False)
    y_out = nc.declare_dram_parameter("y", [N, DIM], F32, isOutput=True)
    sc_out = nc.declare_dram_parameter("score", [1, N], F32, isOutput=True)

    with tile.TileContext(nc) as tc:
        with tc.tile_pool(name="cst", bufs=1) as cst, \
             tc.tile_pool(name="stage", bufs=1) as stage, \
             tc.tile_pool(name="wrk", bufs=3) as wrk, \
             tc.tile_pool(name="wrkta", bufs=2) as wrkta, \
             tc.tile_pool(name="wrk4", bufs=4) as wrk4, \
             tc.tile_pool(name="zp", bufs=2) as zp, \
             tc.tile_pool(name="eph", bufs=2) as eph, \
             tc.tile_pool(name="one", bufs=1) as one, \
             tc.tile_pool(name="ppA", bufs=1, space="PSUM") as ppA, \
             tc.tile_pool(name="ppB", bufs=1, space="PSUM") as ppB, \
             tc.tile_pool(name="pvA", bufs=1, space="PSUM") as pvA, \
             tc.tile_pool(name="pvB", bufs=1, space="PSUM") as pvB:

            # ---- loads ----
            xt_t = []
            wq_t = []
            for kt in range(4):
                xk = stage.tile([128, N], F32, tag=f"xt{kt}")
                nc.sync.dma_start(out=xk, in_=xT[kt * 128:(kt + 1) * 128, :])
                wk = stage.tile([128, 2 * INNER], F32, tag=f"wq{kt}")
                nc.sync.dma_start(out=wk[:, 0:512],
                                  in_=wqk[kt * 128:(kt + 1) * 128, 0:512])
                nc.sync.dma_start(out=wk[:, 512:1024],
                                  in_=wqk[kt * 128:(kt + 1) * 128, 512:1024])
                xt_t.append(xk)
                wq_t.append(wk)
            xtb = cst.tile([128, 4, N], BF16)
            nc.sync.dma_start(out=xtb, in_=xTbf[:, :].rearrange("(t p) i -> p t i", p=128))
            msk = cst.tile([128, 8, N], BF16)
            nc.sync.dma_start(out=msk, in_=maskT[:, :].rearrange("(t p) i -> p t i", p=128))
            wvb = cst.tile([128, 4, INNER], BF16)
            nc.sync.dma_start(out=wvb, in_=wvbf[:, :].rearrange("(t p) c -> p t c", p=128))
            wob = cst.tile([128, 4, DIM], BF16)
            nc.sync.dma_start(out=wob, in_=wobf[:, :].rearrange("(t p) e -> p t e", p=128))
            bb = cst.tile([128, DIM], F32)
            nc.sync.dma_start(out=bb, in_=bout[0:1, :].to_broadcast([128, DIM]))

            ones32 = cst.tile([128, 1], F32)
            nc.vector.memset(ones32, 1.0)
            onesbf = cst.tile([128, 1], BF16)
            nc.vector.memset(onesbf, 1.0)

            vv = cst.tile([128, HEADS, 8, DH + 1], BF16)
            nc.vector.memset(vv[:, :, :, DH:DH + 1], 1.0)

            qkT = cst.tile([128, 8, N], F32)
            onorm = cst.tile([128, 4, N], BF16)

            # ---- QK projection (fp32) ----
            for ct in range(8):
                pool, tag = (ppA, "dA") if ct % 2 == 0 else (ppB, "dB")
                pq = pool.tile([128, N], F32, tag=tag)
                for ic in range(2):
                    sl = slice(ic * 512, (ic + 1) * 512)
                    for kt in range(4):
                        nc.tensor.matmul(
                            pq[:, sl],
                            wq_t[kt][:, ct * 128:(ct + 1) * 128],
                            xt_t[kt][:, sl],
                            start=(kt == 0), stop=(kt == 3),
                            skip_group_check=True)
                nc.scalar.activation(out=qkT[:, ct, :], in_=pq, func=AFT.Copy)

            # score accumulator tiles: reuse xt/wq stage slots (dead after qkproj)
            A_t = []
            for j in range(8):
                atag = f"xt{j}" if j < 4 else f"wq{j - 4}"
                ajt = stage.tile([128, N], F32, tag=atag, name=f"A{j}")
                A_t.append(ajt)

            # ---- V projection (bf16) ----
            for jt in range(8):
                pool, tag = (ppA, "dA") if jt % 2 == 0 else (ppB, "dB")
                pv = pool.tile([128, INNER], F32, tag=tag)
                for kt in range(4):
                    nc.tensor.matmul(
                        pv,
                        xtb[:, kt, jt * 128:(jt + 1) * 128],
                        wvb[:, kt, :],
                        start=(kt == 0), stop=(kt == 3))
                nc.scalar.activation(
                    out=vv[:, :, jt, 0:DH],
                    in_=pv.rearrange("p (h d) -> p h d", h=HEADS),
                    func=AFT.Copy)

            # ---- nnz ----
            nzp = pvA.tile([1, N], F32, tag="vA")
            for jt in range(8):
                for ic in range(2):
                    sl = slice(ic * 512, (ic + 1) * 512)
                    nc.tensor.matmul(nzp[:, sl], onesbf, msk[:, jt, sl],
                                     start=(jt == 0), stop=(jt == 7),
                                     skip_group_check=True)
            scr = one.tile([1, N], F32, tag="scr")
            rnz = one.tile([1, N], F32, tag="rnz")
            nc.vector.reciprocal_approx_accurate(out=rnz, in_=nzp, scratch=scr)

            # ---- attention, per head ----
            for h in range(HEADS):
                po = (h % 2) * 64
                qct, kct = h // 2, 4 + h // 2
                avpool, avtag = (pvA, "vA") if h % 2 == 0 else (pvB, "vB")
                av = avpool.tile([DH + 1, N], F32, tag=avtag)
                for jt in range(8):
                    dpool, dtag = (ppA, "dA") if jt % 2 == 0 else (ppB, "dB")
                    d = dpool.tile([128, N], F32, tag=dtag)
                    for ic in range(2):
                        sl = slice(ic * 512, (ic + 1) * 512)
                        nc.tensor.matmul(
                            d[:, sl],
                            qkT[po:po + 64, kct, jt * 128:(jt + 1) * 128],
                            qkT[po:po + 64, qct, sl],
                            start=True, stop=True, tile_position=(po, 0),
                            skip_group_check=True)
                    t = wrk.tile([128, N], F32, tag="t")
                    nc.vector.tensor_tensor(out=t, in0=d, in1=msk[:, jt, :],
                                            op=AOP.mult)
                    es = wrk4.tile([128, N], BF16, tag="e")
                    nc.scalar.activation(out=es, in_=t, func=AFT.Exp, scale=SCALE)
                    if h == 0:
                        nc.scalar.activation(out=A_t[jt], in_=t, func=AFT.Abs)
                    else:
                        ta = wrkta.tile([128, N], F32, tag="ta")
                        if h in (1, 2):
                            nc.vector.tensor_scalar(
                                out=ta.bitcast(U32), in0=t.bitcast(U32),
                                scalar1=0x7FFFFFFF, scalar2=None,
                                op0=AOP.bitwise_and)
                        else:
                            nc.scalar.activation(out=ta, in_=t, func=AFT.Abs)
                        nc.gpsimd.tensor_tensor(out=A_t[jt], in0=ta,
                                                in1=A_t[jt], op=AOP.add)
                    for ic in range(2):
                        sl = slice(ic * 512, (ic + 1) * 512)
                        nc.tensor.matmul(av[:, sl], vv[:, h, jt, :], es[:, sl],
                                         start=(jt == 0), stop=(jt == 7),
                                         skip_group_check=True)
                # softmax denominator: PSUM row 64 -> SBUF, 1/Z, broadcast
                zrow = zp.tile([1, N], F32, tag="zrow")
                nc.scalar.activation(out=zrow, in_=av[DH:DH + 1, :], func=AFT.Copy)
                zr = zp.tile([1, N], F32, tag="zr")
                nc.vector.reciprocal_approx_fast(out=zr, in_=zrow)
                zbh = zp.tile([128, N], F32, tag="zb")
                nc.gpsimd.partition_broadcast(zbh, zr)
                nc.vector.tensor_tensor(
                    out=onorm[po:po + 64, qct, :], in0=av[0:DH, :],
                    in1=zbh[po:po + 64, :], op=AOP.mult)

            # ---- score: masked abs-sum already in A; reduce over keys ----
            scp = ppA.tile([1, N], F32, tag="dA")
            for jt in range(8):
                for ic in range(2):
                    sl = slice(ic * 512, (ic + 1) * 512)
                    nc.tensor.matmul(scp[:, sl], ones32, A_t[jt][:, sl],
                                     start=(jt == 0), stop=(jt == 7),
                                     skip_group_check=True)
            sc_sb = one.tile([1, N], F32, tag="scr")
            nc.vector.scalar_tensor_tensor(
                out=sc_sb, in0=scp, scalar=SCALE, in1=rnz,
                op0=AOP.mult, op1=AOP.mult)
            nc.gpsimd.dma_start(out=sc_out[:, :], in_=sc_sb)

            # ---- output projection (bf16) ----
            for it in range(8):
                pool, tag = (ppB, "dB") if it % 2 == 0 else (ppA, "dA")
                yp = pool.tile([128, DIM], F32, tag=tag)
                for pr in range(4):
                    nc.tensor.matmul(
                        yp,
                        onorm[:, pr, it * 128:(it + 1) * 128],
                        wob[:, pr, :],
                        start=(pr == 0), stop=(pr == 3))
                yt = eph.tile([128, DIM], F32, tag="yt")
                nc.vector.tensor_tensor(out=yt, in0=yp, in1=bb, op=AOP.add)
                nc.sync.dma_start(out=y_out[it * 128:(it + 1) * 128, :], in_=yt)
    nc.finalize()
    return nc


def _get_nc():
    if "nc" not in _cache:
        _cache["nc"] = _build()
    return _cache["nc"]


def _run_device(inputs, trace=False):
    x = np.asarray(inputs["x"], np.float32)
    cp_mask = np.asarray(inputs["cp_mask"])
    w_qkv = np.asarray(inputs["w_qkv"], np.float32)
    w_out = np.asarray(inputs["w_out"], np.float32)
    b_out = np.asarray(inputs["b_out"], np.float32)

    bf = mybir.dt.np(BF16)
    maskT = np.ascontiguousarray(cp_mask.T).astype(bf)
    wqk = np.ascontiguousarray(w_qkv[:, :2 * INNER])
    wvbf = np.ascontiguousarray(w_qkv[:, 2 * INNER:]).astype(bf)
    wobf = np.ascontiguousarray(w_out).astype(bf)
    boutr = np.ascontiguousarray(b_out.reshape(1, DIM))

    in_maps = []
    for b in range(B):
        xTb = np.ascontiguousarray(x[b].T)
        in_maps.append({
            "xT": xTb,
            "xTbf": xTb.astype(bf),
            "maskT": maskT,
            "wqk": wqk,
            "wvbf": wvbf,
            "wobf": wobf,
            "bout": boutr,
        })

    nc = _get_nc()
    res = run_bass_kernel_spmd(nc, in_maps, core_ids=list(range(B)), trace=trace)
    y = np.stack([res.results[b]["y"] for b in range(B)])
    score = np.stack([res.results[b]["score"][0] for b in range(B)])
    return y, score, res


def _apply_swap(y, score, patches):
    idx = np.argsort(score, axis=-1, kind="stable")[::-1]
    out = y.copy()
    clone = y
    bi = np.arange(B)
    for i in range(1, patches + 1):
        ti = idx[:, i]
        out[bi, i] = clone[bi, ti]
        out[bi, ti] = clone[:, i]
    return out


def kernel(**inputs):
    patches = int(np.asarray(inputs["patches_in_core_nodes"]))
    y, score, _ = _run_device(inputs, trace=False)
    return _apply_swap(y, score, patches)
